# revision 28
# baseline (speedup 1.0000x reference)
"""Trainium2 Bass kernel for nn_BasicBlock (per-sample dynamic 3x3 convs +
sync-BN + residual ReLU), data-parallel over batch on 8 NeuronCores.

Reference semantics (B=16, C=64, H=W=128):
    out = relu(bn2(conv2(relu(bn1(conv1(x, f1))), f2)) + x)
with training-mode BN over full-batch (N,H,W) statistics.

Sharding: 2 samples per core. Per-sample convs become block-diagonal
128x128 matmuls (partitions 0-63 = sample A channels, 64-127 = sample B).
BN batch statistics are made exact via a tiny AllReduce of per-channel
(mean, E[x^2]) sums over the 16 (sample, core) groups.

Convs run as 9 shifted-tap matmuls per spatial tile ([128,512] PSUM
accumulation) against a zero-padded image held in SBUF. Matmul inputs are
bf16; accumulation is fp32. Raw conv outputs are staged bf16 for BN,
normalization math is fp32.

v2 layout of work:
  - gpsimd queue carries ONLY the collectives (instant triggers).
  - x input streamed on sync+vector queues, weights first on scalar.
  - BN params chain: AllReduce [mu, E[x^2]] sums, sqrt+reciprocal (no
    Newton), ~9 small ops.
  - final phase: psum_t = I@x_t (pre-AR2) + diag(a2)@y2_t on the idle
    tensor engine, then ONE fused bias+relu pass per tile, alternating
    ACT/DVE; output DMA'd bf16 in 8 chunks on sync.
  - filler matmuls keep the PE HAM-warm across both AllReduce gaps.
"""
import numpy as np

import concourse.bass as bass
import concourse.mybir as mybir
import concourse.tile as tile
from concourse import bacc
from concourse.bass_utils import run_bass_kernel_spmd

N_CORES = 8
B, C, H, W = 16, 64, 128, 128
SPC = B // N_CORES            # samples per core (2)
HP, WP = H + 2, W + 2         # padded image
TR = 4                        # image rows per spatial tile
NT = H // TR                  # 32 tiles
N = TR * W                    # 512 moving elements per matmul
NGROUPS = B                   # 16 (sample, core) stat groups of H*W each
BN_EPS = 1e-5
N_WARM = 24                   # readback-gated tiny PE warmers per AR gap

F32 = mybir.dt.float32
BF16 = mybir.dt.bfloat16
AF = mybir.ActivationFunctionType
ALU = mybir.AluOpType

_CACHE = {}


def _build():
    nc = bacc.Bacc("TRN2", target_bir_lowering=False, debug=False,
                   num_devices=N_CORES)
    xp_ext = nc.dram_tensor("xp", [128, HP, WP], BF16, kind="ExternalInput").ap()
    w_ext = nc.dram_tensor("w", [128, 2, 9, 128], BF16, kind="ExternalInput").ap()
    eye_ext = nc.dram_tensor("eye", [128, 128], BF16, kind="ExternalInput").ap()
    cst_ext = nc.dram_tensor("cst", [128, 4], F32, kind="ExternalInput").ap()
    out_ext = nc.dram_tensor("out", [128, H, W], BF16, kind="ExternalOutput").ap()

    with tile.TileContext(nc) as tc:
        with tc.tile_pool(name="sb", bufs=1) as sb, \
             tc.tile_pool(name="ps", bufs=8, space="PSUM") as ps, \
             tc.tile_pool(name="dram", bufs=1, space="DRAM") as dram:

            x_pad = sb.tile([128, HP * WP], BF16, tag="x_pad")
            norm_pad = sb.tile([128, HP * WP], BF16, tag="norm_pad")
            raw = sb.tile([128, H * W], BF16, tag="raw")
            fin = sb.tile([128, H * W], BF16, tag="fin")
            wsb = sb.tile([128, 2 * 9 * 128], BF16, tag="wsb")
            eye = sb.tile([128, 128], BF16, tag="eye")
            diag = sb.tile([128, 128], BF16, tag="diag")
            cst = sb.tile([128, 4], F32, tag="cst")
            st6 = [sb.tile([128, NT * 6], F32, tag=f"st6_{c}", name=f"st6_{c}")
                   for c in range(2)]
            gst = sb.tile([128, 2 * 2], F32, tag="gst")      # [k=2, s=2]
            params = sb.tile([128, 4], F32, tag="params")    # a1 b1 a2 b2
            sml = sb.tile([128, 32], F32, tag="sml")         # small scratch
            mvt = sb.tile([128, 32], F32, tag="mvt")         # stats pre-transpose
            trs = sb.tile([128, 32], F32, tag="trs")         # stats transposed

            cc_in = [dram.tile([128 * 2], F32, name=f"cc_in{c}") for c in range(2)]
            cc_out = [dram.tile([128 * 2], F32, name=f"cc_out{c}") for c in range(2)]

            warm_in = dram.tile([128 * 2], F32)     # shape-matches the BN ARs
            warm_out = dram.tile([128 * 2], F32)

            x3 = x_pad.rearrange("p (h w) -> p h w", h=HP)
            n3 = norm_pad.rearrange("p (h w) -> p h w", h=HP)
            wv = wsb.rearrange("p (c t m) -> p c t m", c=2, t=9)

            # warmup collective: pre-pays the cold-first-collective cost on the
            # CC stream whenever the entry barrier finishes before conv1 does
            nc.gpsimd.collective_compute(
                "AllReduce", ALU.add,
                replica_groups=[list(range(N_CORES))],
                ins=[warm_in.opt()], outs=[warm_out.opt()])

            # weights first on scalar (first MM needs conv1 taps only)
            wflat = w_ext.rearrange("k c t m -> k c (t m)")
            wv_sb = wsb.rearrange("p (c tm) -> p c tm", c=2)
            nc.scalar.dma_start(out=wv_sb[:, 0], in_=wflat[:, 0])
            nc.scalar.dma_start(out=wv_sb[:, 1], in_=wflat[:, 1])

            # norm_pad borders <- zeros (conv2's padding) via DVE memsets
            nc.vector.memset(sml[:, 0:1], 0.0)
            nc.vector.memset(sml[:, 30:31], BN_EPS)      # eps bias for Sqrt
            nc.vector.memset(n3[:, 0, :], 0.0)
            nc.vector.memset(n3[:, HP - 1, :], 0.0)
            nc.vector.memset(n3[:, :, 0], 0.0)
            nc.vector.memset(n3[:, :, WP - 1], 0.0)

            # x (pre-padded on host), fine-to-coarse chunks; two of the middle
            # chunks ride scalar behind the weights, the rest stream on sync
            x_chunks = [(nc.sync, 0, 6), (nc.sync, 6, 14), (nc.scalar, 14, 26),
                        (nc.sync, 26, 42), (nc.scalar, 42, 62),
                        (nc.sync, 62, 86), (nc.sync, 86, 110),
                        (nc.sync, 110, HP)]
            for eng, r0, r1 in x_chunks:
                eng.dma_start(out=x3[:, r0:r1, :], in_=xp_ext[:, r0:r1, :])

            # consts late (needed only at the BN points), primers after
            nc.scalar.dma_start(out=cst[:, :], in_=cst_ext)
            nc.scalar.dma_start(out=eye[:, :], in_=eye_ext)

            # prime the ACT function-table set holding Relu/Sqrt/Copy so the
            # mid-kernel PSEUDO_LOAD happens here, off the critical path
            nc.scalar.activation(sml[:, 1:2], sml[:, 0:1], AF.Relu)
            nc.scalar.activation(sml[:, 2:3], sml[:, 0:1], AF.Sqrt)

            # ---- conv + stats helper ----
            def conv_tile(src3, conv_idx, st6_t, t, mix_evac):
                psum = ps.tile([128, N], F32, tag="psum")
                r0 = t * TR
                for tap in range(9):
                    kh, kw = tap // 3, tap % 3
                    rhs = src3[:, r0 + kh:r0 + kh + TR, kw:kw + W]
                    nc.tensor.matmul(psum[:, :], wv[:, conv_idx, tap, :], rhs,
                                     start=(tap == 0), stop=(tap == 8))
                # stats first (tail-critical), then evacuation
                nc.vector.bn_stats(st6_t[:, t * 6:(t + 1) * 6], psum[:, :])
                rt = raw[:, t * N:(t + 1) * N]
                if mix_evac and t % 2 == 1:
                    nc.vector.tensor_copy(rt, psum[:, :])
                else:
                    nc.scalar.activation(rt, psum[:, :], AF.Copy)

            # ---- local stats -> [mu/16, E[x^2]/16] -> AllReduce trigger ----
            # The [128, 2] per-partition stats are transposed on-chip so the
            # DRAM payload DMA is 4 contiguous 128B rows (cheap completion)
            # instead of a 128-partition x 8B scatter (~4-6 us completion).
            def stats_to_ar(c):
                mv = mvt[:, 0:2]                           # [mu, var]
                nc.vector.bn_aggr(mv, st6[c].rearrange("p (t k) -> p t k", k=6))
                # var -> E[x^2] = var + mu^2, then scale both by 1/NGROUPS
                nc.vector.scalar_tensor_tensor(mv[:, 1:2], mv[:, 0:1], mv[:, 0:1],
                                               mv[:, 1:2], op0=ALU.mult,
                                               op1=ALU.add)
                nc.vector.tensor_scalar_mul(mv, mv, 1.0 / NGROUPS)
                nc.vector.transpose(trs[:, :], mvt[:, :])
                # row j of each 32-block holds stat j for that channel block:
                # partitions {32b + j}; DMA rows j=0 (mu) and j=1 (E) separately
                # into a (r s bb c5) DRAM layout: 128B-contiguous writes AND
                # only 2 stride-512B reads per partition on the way back
                trv = trs.rearrange("(b j) f -> j b f", b=4)
                nc.sync.dma_start(out=cc_in[c][0:128], in_=trv[0],
                                  single_packet=True)
                nc.scalar.dma_start(out=cc_in[c][128:256], in_=trv[1],
                                    single_packet=True)
                nc.gpsimd.collective_compute(
                    "AllReduce", ALU.add,
                    replica_groups=[list(range(N_CORES))],
                    ins=[cc_in[c].opt()], outs=[cc_out[c].opt()])

            # ---- AR result -> per-channel scale a, bias b ----
            def ar_to_params(c, gamma_ap, beta_ap, a_ap, b_ap):
                # cc layout: [(r s bb c5)] with r=stat, s=sample half,
                # bb=channel-block-of-32, c5=channel-in-block
                src = cc_out[c].rearrange("(r s bb c) -> (bb c) r s",
                                          r=2, s=2, bb=2)
                g3 = gst.rearrange("p (k s) -> p k s", k=2)
                nc.sync.dma_start(out=g3[0:64], in_=src)
                nc.scalar.dma_start(out=g3[64:128], in_=src)
                # tiny matmuls gated on the readback: wake the PE the moment AR
                # results land so the real stream restarts HAM-warm
                fp = ps.tile([128, N], F32, tag="psum", name=f"warm{c}")
                for i in range(N_WARM):
                    nc.tensor.matmul(fp[:, 0:4], wv[:, 0, 0, :],
                                     gst.bitcast(BF16)[:, 0:4],
                                     start=True, stop=True)
                o = 20 + 5 * c
                g2 = sml[:, o:o + 2]                       # [mean, E[x^2]] global
                nc.vector.tensor_reduce(g2, g3, axis=mybir.AxisListType.X,
                                        op=ALU.add)
                m2 = sml[:, o + 2:o + 3]
                nc.vector.tensor_mul(m2, g2[:, 0:1], g2[:, 0:1])        # mean^2
                ve = sml[:, o + 3:o + 4]
                nc.vector.scalar_tensor_tensor(ve, m2, -1.0, g2[:, 1:2],
                                               op0=ALU.mult, op1=ALU.add)  # var
                sd = sml[:, o + 4:o + 5]
                nc.scalar.activation(sd, ve, AF.Sqrt, bias=sml[:, 30:31])
                y0 = sml[:, 2:3]
                nc.vector.reciprocal(y0, sd)
                nc.vector.tensor_mul(a_ap, y0, gamma_ap)
                tmp = sml[:, 3:4]
                nc.vector.tensor_mul(tmp, a_ap, g2[:, 0:1])             # a*mean
                nc.vector.scalar_tensor_tensor(b_ap, tmp, -1.0, beta_ap,
                                               op0=ALU.mult, op1=ALU.add)

            # ---- pipeline ----
            for t in range(NT):
                conv_tile(x3, 0, st6[0], t, mix_evac=False)
            stats_to_ar(0)
            ar_to_params(0, cst[:, 0:1], cst[:, 1:2], params[:, 0:1], params[:, 1:2])

            # norm1: relu(a1*raw + b1) -> norm_pad interior, interleaved with
            # conv2 emission (2 tiles ahead)
            def norm1_tile(t):
                rt = raw[:, t * N:(t + 1) * N].rearrange("p (a b) -> p a b", a=TR)
                dst = n3[:, 1 + t * TR:1 + (t + 1) * TR, 1:1 + W]
                nc.scalar.activation(dst, rt, AF.Relu,
                                     scale=params[:, 0:1], bias=params[:, 1:2])

            norm1_tile(0)
            norm1_tile(1)
            for t in range(NT):
                if t + 2 < NT:
                    norm1_tile(t + 2)
                conv_tile(n3, 1, st6[1], t, mix_evac=True)
            stats_to_ar(1)

            # residual pre-accumulation: psum_t = I @ x_t (no AR dependency)
            def x_tile(t):
                return x3[:, 1 + t * TR:1 + (t + 1) * TR, 1:1 + W]

            fps = []
            for t in range(6):
                p = ps.tile([128, N], F32, tag="psum", name=f"fin_ps{t}")
                nc.tensor.matmul(p[:, :], eye[:, :], x_tile(t),
                                 start=True, stop=False)
                fps.append(p)

            ar_to_params(1, cst[:, 2:3], cst[:, 3:4], params[:, 2:3], params[:, 3:4])
            # diag(a2) as matmul lhsT: eye * a2 (per-partition scalar)
            nc.vector.tensor_scalar_mul(diag[:, :], eye[:, :], params[:, 2:3])

            # final: psum = I@x + diag(a2)@y2; one fused relu(+b2) pass,
            # alternating ACT/DVE; bf16 out DMA'd in 8 chunks on sync.
            # MMs grouped in pairs sharing lhsT (diag,diag,eye,eye) to cut
            # LDWEIGHTS switches.
            for t in range(0, NT, 2):
                for u in (t, t + 1):
                    nc.tensor.matmul(fps[u][:, :], diag[:, :],
                                     raw[:, u * N:(u + 1) * N],
                                     start=False, stop=True)
                for u in (t, t + 1):
                    if u + 6 < NT:
                        p2 = ps.tile([128, N], F32, tag="psum",
                                     name=f"fin_ps{u + 6}")
                        nc.tensor.matmul(p2[:, :], eye[:, :], x_tile(u + 6),
                                         start=True, stop=False)
                        fps.append(p2)
                nc.scalar.activation(fin[:, t * N:(t + 1) * N], fps[t][:, :],
                                     AF.Relu, bias=params[:, 3:4])
                nc.vector.tensor_scalar(fin[:, (t + 1) * N:(t + 2) * N],
                                        fps[t + 1][:, :], params[:, 3:4], 0.0,
                                        op0=ALU.add, op1=ALU.max)
                if t < NT - 4 and t % 4 == 2:
                    t0 = t - 2
                    nc.sync.dma_start(
                        out=out_ext[:, t0 * TR:(t + 2) * TR, :],
                        in_=fin[:, t0 * N:(t + 2) * N].rearrange(
                            "p (h w) -> p h w", w=W))
            # last 4 tiles in two 2-tile chunks on separate queues to shrink
            # the end-of-kernel DMA drain
            for t0, eng in ((28, nc.scalar), (30, nc.sync)):
                eng.dma_start(
                    out=out_ext[:, t0 * TR:(t0 + 2) * TR, :],
                    in_=fin[:, t0 * N:(t0 + 2) * N].rearrange(
                        "p (h w) -> p h w", w=W))

    nc.compile()
    return nc


def _get_nc():
    if "nc" not in _CACHE:
        _CACHE["nc"] = _build()
    return _CACHE["nc"]


def _pack_inputs(x, filters1, filters2, gamma1, beta1, gamma2, beta2):
    import ml_dtypes
    bf = ml_dtypes.bfloat16
    x = np.ascontiguousarray(x, dtype=np.float32)
    in_maps = []
    gb = np.stack([np.tile(np.asarray(g, np.float32), 2) for g in
                   (gamma1, beta1, gamma2, beta2)], axis=1)  # [128, 4]
    eye = np.eye(128, dtype=bf)
    for i in range(N_CORES):
        s0, s1 = SPC * i, SPC * i + 1
        xp = np.zeros((128, HP, WP), bf)
        xp[0:C, 1:1 + H, 1:1 + W] = x[s0]
        xp[C:128, 1:1 + H, 1:1 + W] = x[s1]
        w = np.zeros((128, 2, 9, 128), bf)
        for ci, f in enumerate((filters1, filters2)):
            f = np.asarray(f, np.float32)
            # w[k, ci, tap, m]: lhsT[k=cin, m=cout], block-diagonal over samples
            fs0 = f[s0].transpose(1, 2, 3, 0).reshape(C, 9, C)   # [cin, tap, cout]
            fs1 = f[s1].transpose(1, 2, 3, 0).reshape(C, 9, C)
            w[0:C, ci, :, 0:C] = fs0
            w[C:128, ci, :, C:128] = fs1
        in_maps.append({"xp": xp, "w": w, "cst": gb, "eye": eye})
    return in_maps


def _run(in_maps, trace=False):
    nc = _get_nc()
    return run_bass_kernel_spmd(nc, in_maps, core_ids=list(range(N_CORES)),
                                trace=trace)


def _gather(res):
    out = np.empty((B, C, H, W), np.float32)
    for i in range(N_CORES):
        o = np.asarray(res.results[i]["out"], dtype=np.float32)
        out[SPC * i] = o[0:C]
        out[SPC * i + 1] = o[C:128]
    return out


def kernel(x, filters1, filters2, gamma1, beta1, gamma2, beta2):
    in_maps = _pack_inputs(x, filters1, filters2, gamma1, beta1, gamma2, beta2)
    res = _run(in_maps, trace=False)
    return _gather(res)


# revision 29
# speedup vs baseline: 1.2099x; 1.2099x over previous
"""Trainium2 Bass kernel for nn_BasicBlock (per-sample dynamic 3x3 convs +
sync-BN + residual ReLU), data-parallel over batch on 8 NeuronCores.

Reference semantics (B=16, C=64, H=W=128):
    out = relu(bn2(conv2(relu(bn1(conv1(x, f1))), f2)) + x)
with training-mode BN over full-batch (N,H,W) statistics.

Sharding: 2 samples per core. Per-sample convs become block-diagonal
128x128 matmuls (partitions 0-63 = sample A channels, 64-127 = sample B).
BN batch statistics are made exact via a tiny AllReduce of per-channel
(mean, E[x^2]) sums over the 16 (sample, core) groups.

Convs run as 9 shifted-tap matmuls per spatial tile ([128,512] PSUM
accumulation) against a zero-padded image held in SBUF. Matmul inputs are
bf16; accumulation is fp32. Raw conv outputs are staged bf16 for BN,
normalization math is fp32.

v2 layout of work:
  - gpsimd queue carries ONLY the collectives (instant triggers).
  - x input streamed on sync+vector queues, weights first on scalar.
  - BN params chain: AllReduce [mu, E[x^2]] sums, sqrt+reciprocal (no
    Newton), ~9 small ops.
  - final phase: psum_t = I@x_t (pre-AR2) + diag(a2)@y2_t on the idle
    tensor engine, then ONE fused bias+relu pass per tile, alternating
    ACT/DVE; output DMA'd bf16 in 8 chunks on sync.
  - filler matmuls keep the PE HAM-warm across both AllReduce gaps.
"""
import numpy as np

import concourse.bass as bass
import concourse.mybir as mybir
import concourse.tile as tile
from concourse import bacc
from concourse.bass_utils import run_bass_kernel_spmd

N_CORES = 8
B, C, H, W = 16, 64, 128, 128
SPC = B // N_CORES            # samples per core (2)
HP, WP = H + 2, W + 2         # padded image
TR = 4                        # image rows per spatial tile
NT = H // TR                  # 32 tiles
N = TR * W                    # 512 moving elements per matmul
NGROUPS = B                   # 16 (sample, core) stat groups of H*W each
BN_EPS = 1e-5
N_WARM = 24                   # readback-gated tiny PE warmers per AR gap

F32 = mybir.dt.float32
BF16 = mybir.dt.bfloat16
AF = mybir.ActivationFunctionType
ALU = mybir.AluOpType

_CACHE = {}


def _build():
    nc = bacc.Bacc("TRN2", target_bir_lowering=False, debug=False,
                   num_devices=N_CORES)
    xp_ext = nc.dram_tensor("xp", [128, HP, WP], BF16, kind="ExternalInput").ap()
    w_ext = nc.dram_tensor("w", [128, 2, 9, 128], BF16, kind="ExternalInput").ap()
    eye_ext = nc.dram_tensor("eye", [128, 128], BF16, kind="ExternalInput").ap()
    cst_ext = nc.dram_tensor("cst", [128, 4], F32, kind="ExternalInput").ap()
    out_ext = nc.dram_tensor("out", [128, H, W], BF16, kind="ExternalOutput").ap()

    with tile.TileContext(nc) as tc:
        with tc.tile_pool(name="sb", bufs=1) as sb, \
             tc.tile_pool(name="ps", bufs=8, space="PSUM") as ps, \
             tc.tile_pool(name="dram", bufs=1, space="DRAM") as dram:

            x_pad = sb.tile([128, HP * WP], BF16, tag="x_pad")
            norm_pad = sb.tile([128, HP * WP], BF16, tag="norm_pad")
            raw = sb.tile([128, H * W], BF16, tag="raw")
            fin = sb.tile([128, H * W], BF16, tag="fin")
            wsb = sb.tile([128, 2 * 9 * 128], BF16, tag="wsb")
            eye = sb.tile([128, 128], BF16, tag="eye")
            diag = sb.tile([128, 128], BF16, tag="diag")
            cst = sb.tile([128, 4], F32, tag="cst")
            st6 = [sb.tile([128, NT * 6], F32, tag=f"st6_{c}", name=f"st6_{c}")
                   for c in range(2)]
            gst = sb.tile([128, 2 * 2], F32, tag="gst")      # [k=2, s=2]
            params = sb.tile([128, 4], F32, tag="params")    # a1 b1 a2 b2
            sml = sb.tile([128, 32], F32, tag="sml")         # small scratch
            mvt = sb.tile([128, 32], F32, tag="mvt")         # stats pre-transpose
            trs = sb.tile([128, 32], F32, tag="trs")         # stats transposed

            cc_in = [dram.tile([128 * 2], F32, name=f"cc_in{c}") for c in range(2)]
            cc_out = [dram.tile([128 * 2], F32, name=f"cc_out{c}") for c in range(2)]

            warm_in = dram.tile([128 * 2], F32)     # shape-matches the BN ARs
            warm_out = dram.tile([128 * 2], F32)

            x3 = x_pad.rearrange("p (h w) -> p h w", h=HP)
            n3 = norm_pad.rearrange("p (h w) -> p h w", h=HP)
            wv = wsb.rearrange("p (c t m) -> p c t m", c=2, t=9)

            # warmup collective: pre-pays the cold-first-collective cost on the
            # CC stream whenever the entry barrier finishes before conv1 does
            nc.gpsimd.collective_compute(
                "AllReduce", ALU.add,
                replica_groups=[list(range(N_CORES))],
                ins=[warm_in.opt()], outs=[warm_out.opt()])

            # weights first on scalar (first MM needs conv1 taps only)
            wflat = w_ext.rearrange("k c t m -> k c (t m)")
            wv_sb = wsb.rearrange("p (c tm) -> p c tm", c=2)
            nc.scalar.dma_start(out=wv_sb[:, 0], in_=wflat[:, 0])
            nc.scalar.dma_start(out=wv_sb[:, 1], in_=wflat[:, 1])

            # norm_pad borders <- zeros (conv2's padding) via DVE memsets
            nc.vector.memset(sml[:, 0:1], 0.0)
            nc.vector.memset(sml[:, 30:31], BN_EPS)      # eps bias for Sqrt
            nc.vector.memset(n3[:, 0, :], 0.0)
            nc.vector.memset(n3[:, HP - 1, :], 0.0)
            nc.vector.memset(n3[:, :, 0], 0.0)
            nc.vector.memset(n3[:, :, WP - 1], 0.0)

            # x (pre-padded on host), fine-to-coarse chunks; two of the middle
            # chunks ride scalar behind the weights, the rest stream on sync
            x_chunks = [(nc.sync, 0, 6), (nc.sync, 6, 14), (nc.scalar, 14, 26),
                        (nc.sync, 26, 42), (nc.scalar, 42, 62),
                        (nc.sync, 62, 86), (nc.sync, 86, 110),
                        (nc.sync, 110, HP)]
            for eng, r0, r1 in x_chunks:
                eng.dma_start(out=x3[:, r0:r1, :], in_=xp_ext[:, r0:r1, :])

            # consts late (needed only at the BN points), primers after
            nc.scalar.dma_start(out=cst[:, :], in_=cst_ext)
            nc.scalar.dma_start(out=eye[:, :], in_=eye_ext)

            # prime the ACT function-table set holding Relu/Sqrt/Copy so the
            # mid-kernel PSEUDO_LOAD happens here, off the critical path
            nc.scalar.activation(sml[:, 1:2], sml[:, 0:1], AF.Relu)
            nc.scalar.activation(sml[:, 2:3], sml[:, 0:1], AF.Sqrt)

            # ---- conv + stats helper ----
            def conv_tile(src3, conv_idx, st6_t, t, mix_evac):
                psum = ps.tile([128, N], F32, tag="psum")
                r0 = t * TR
                for tap in range(9):
                    kh, kw = tap // 3, tap % 3
                    rhs = src3[:, r0 + kh:r0 + kh + TR, kw:kw + W]
                    nc.tensor.matmul(psum[:, :], wv[:, conv_idx, tap, :], rhs,
                                     start=(tap == 0), stop=(tap == 8))
                # stats first (tail-critical), then evacuation
                nc.vector.bn_stats(st6_t[:, t * 6:(t + 1) * 6], psum[:, :])
                rt = raw[:, t * N:(t + 1) * N]
                if mix_evac and t % 2 == 1:
                    nc.vector.tensor_copy(rt, psum[:, :])
                else:
                    nc.scalar.activation(rt, psum[:, :], AF.Copy)

            # ---- local stats -> [mu/16, E[x^2]/16] -> AllReduce trigger ----
            # The [128, 2] per-partition stats are transposed on-chip so the
            # DRAM payload DMA is 4 contiguous 128B rows (cheap completion)
            # instead of a 128-partition x 8B scatter (~4-6 us completion).
            def stats_to_ar(c):
                mv = mvt[:, 0:2]                           # [mu, var]
                nc.vector.bn_aggr(mv, st6[c].rearrange("p (t k) -> p t k", k=6))
                # var -> E[x^2] = var + mu^2, then scale both by 1/NGROUPS
                nc.vector.scalar_tensor_tensor(mv[:, 1:2], mv[:, 0:1], mv[:, 0:1],
                                               mv[:, 1:2], op0=ALU.mult,
                                               op1=ALU.add)
                nc.vector.tensor_scalar_mul(mv, mv, 1.0 / NGROUPS)
                nc.vector.transpose(trs[:, :], mvt[:, :])
                # row j of each 32-block holds stat j for that channel block:
                # partitions {32b + j}; DMA rows j=0 (mu) and j=1 (E) separately
                # into a (r s bb c5) DRAM layout: 128B-contiguous writes AND
                # only 2 stride-512B reads per partition on the way back
                trv = trs.rearrange("(b j) f -> j b f", b=4)
                nc.sync.dma_start(out=cc_in[c][0:128], in_=trv[0],
                                  single_packet=True)
                nc.scalar.dma_start(out=cc_in[c][128:256], in_=trv[1],
                                    single_packet=True)
                nc.gpsimd.collective_compute(
                    "AllReduce", ALU.add,
                    replica_groups=[list(range(N_CORES))],
                    ins=[cc_in[c].opt()], outs=[cc_out[c].opt()])

            # ---- AR result -> per-channel scale a, bias b ----
            def ar_to_params(c, gamma_ap, beta_ap, a_ap, b_ap):
                # cc layout: [(r s bb c5)] with r=stat, s=sample half,
                # bb=channel-block-of-32, c5=channel-in-block
                src = cc_out[c].rearrange("(r s bb c) -> (bb c) r s",
                                          r=2, s=2, bb=2)
                g3 = gst.rearrange("p (k s) -> p k s", k=2)
                nc.sync.dma_start(out=g3[0:64], in_=src)
                nc.scalar.dma_start(out=g3[64:128], in_=src)
                # tiny matmuls gated on the readback: wake the PE the moment AR
                # results land so the real stream restarts HAM-warm
                fp = ps.tile([128, N], F32, tag="psum", name=f"warm{c}")
                for i in range(N_WARM):
                    nc.tensor.matmul(fp[:, 0:4], wv[:, 0, 0, :],
                                     gst.bitcast(BF16)[:, 0:4],
                                     start=True, stop=True)
                o = 20 + 5 * c
                g2 = sml[:, o:o + 2]                       # [mean, E[x^2]] global
                nc.vector.tensor_reduce(g2, g3, axis=mybir.AxisListType.X,
                                        op=ALU.add)
                m2 = sml[:, o + 2:o + 3]
                nc.vector.tensor_mul(m2, g2[:, 0:1], g2[:, 0:1])        # mean^2
                ve = sml[:, o + 3:o + 4]
                nc.vector.scalar_tensor_tensor(ve, m2, -1.0, g2[:, 1:2],
                                               op0=ALU.mult, op1=ALU.add)  # var
                sd = sml[:, o + 4:o + 5]
                nc.scalar.activation(sd, ve, AF.Sqrt, bias=sml[:, 30:31])
                y0 = sml[:, 2:3]
                nc.vector.reciprocal(y0, sd)
                nc.vector.tensor_mul(a_ap, y0, gamma_ap)
                tmp = sml[:, 3:4]
                nc.vector.tensor_mul(tmp, a_ap, g2[:, 0:1])             # a*mean
                nc.vector.scalar_tensor_tensor(b_ap, tmp, -1.0, beta_ap,
                                               op0=ALU.mult, op1=ALU.add)

            # ---- pipeline ----
            for t in range(NT):
                conv_tile(x3, 0, st6[0], t, mix_evac=False)
            stats_to_ar(0)
            ar_to_params(0, cst[:, 0:1], cst[:, 1:2], params[:, 0:1], params[:, 1:2])

            # norm1: relu(a1*raw + b1) -> norm_pad interior, interleaved with
            # conv2 emission (2 tiles ahead)
            def norm1_tile(t):
                rt = raw[:, t * N:(t + 1) * N].rearrange("p (a b) -> p a b", a=TR)
                dst = n3[:, 1 + t * TR:1 + (t + 1) * TR, 1:1 + W]
                nc.scalar.activation(dst, rt, AF.Relu,
                                     scale=params[:, 0:1], bias=params[:, 1:2])

            norm1_tile(0)
            norm1_tile(1)
            for t in range(NT):
                if t + 2 < NT:
                    norm1_tile(t + 2)
                conv_tile(n3, 1, st6[1], t, mix_evac=True)
            stats_to_ar(1)

            # residual pre-accumulation: psum_t = I @ x_t (no AR dependency)
            def x_tile(t):
                return x3[:, 1 + t * TR:1 + (t + 1) * TR, 1:1 + W]

            fps = []
            for t in range(6):
                p = ps.tile([128, N], F32, tag="psum", name=f"fin_ps{t}")
                nc.tensor.matmul(p[:, :], eye[:, :], x_tile(t),
                                 start=True, stop=False)
                fps.append(p)

            ar_to_params(1, cst[:, 2:3], cst[:, 3:4], params[:, 2:3], params[:, 3:4])
            # diag(a2) as matmul lhsT: eye * a2 (per-partition scalar)
            nc.vector.tensor_scalar_mul(diag[:, :], eye[:, :], params[:, 2:3])

            # final: psum = I@x + diag(a2)@y2; one fused relu(+b2) pass,
            # alternating ACT/DVE; bf16 out DMA'd in 8 chunks on sync.
            # MMs grouped in pairs sharing lhsT (diag,diag,eye,eye) to cut
            # LDWEIGHTS switches.
            for t in range(0, NT, 2):
                for u in (t, t + 1):
                    nc.tensor.matmul(fps[u][:, :], diag[:, :],
                                     raw[:, u * N:(u + 1) * N],
                                     start=False, stop=True)
                for u in (t, t + 1):
                    if u + 6 < NT:
                        p2 = ps.tile([128, N], F32, tag="psum",
                                     name=f"fin_ps{u + 6}")
                        nc.tensor.matmul(p2[:, :], eye[:, :], x_tile(u + 6),
                                         start=True, stop=False)
                        fps.append(p2)
                nc.scalar.activation(fin[:, t * N:(t + 1) * N], fps[t][:, :],
                                     AF.Relu, bias=params[:, 3:4])
                nc.vector.tensor_scalar(fin[:, (t + 1) * N:(t + 2) * N],
                                        fps[t + 1][:, :], params[:, 3:4], 0.0,
                                        op0=ALU.add, op1=ALU.max)
                if t < NT - 4 and t % 4 == 2:
                    t0 = t - 2
                    nc.sync.dma_start(
                        out=out_ext[:, t0 * TR:(t + 2) * TR, :],
                        in_=fin[:, t0 * N:(t + 2) * N].rearrange(
                            "p (h w) -> p h w", w=W))
            # last 4 tiles in two 2-tile chunks on separate queues to shrink
            # the end-of-kernel DMA drain
            for t0, eng in ((28, nc.gpsimd), (30, nc.sync)):
                eng.dma_start(
                    out=out_ext[:, t0 * TR:(t0 + 2) * TR, :],
                    in_=fin[:, t0 * N:(t0 + 2) * N].rearrange(
                        "p (h w) -> p h w", w=W))

    nc.compile()
    return nc


def _get_nc():
    if "nc" not in _CACHE:
        _CACHE["nc"] = _build()
    return _CACHE["nc"]


def _pack_inputs(x, filters1, filters2, gamma1, beta1, gamma2, beta2):
    import ml_dtypes
    bf = ml_dtypes.bfloat16
    x = np.ascontiguousarray(x, dtype=np.float32)
    in_maps = []
    gb = np.stack([np.tile(np.asarray(g, np.float32), 2) for g in
                   (gamma1, beta1, gamma2, beta2)], axis=1)  # [128, 4]
    eye = np.eye(128, dtype=bf)
    for i in range(N_CORES):
        s0, s1 = SPC * i, SPC * i + 1
        xp = np.zeros((128, HP, WP), bf)
        xp[0:C, 1:1 + H, 1:1 + W] = x[s0]
        xp[C:128, 1:1 + H, 1:1 + W] = x[s1]
        w = np.zeros((128, 2, 9, 128), bf)
        for ci, f in enumerate((filters1, filters2)):
            f = np.asarray(f, np.float32)
            # w[k, ci, tap, m]: lhsT[k=cin, m=cout], block-diagonal over samples
            fs0 = f[s0].transpose(1, 2, 3, 0).reshape(C, 9, C)   # [cin, tap, cout]
            fs1 = f[s1].transpose(1, 2, 3, 0).reshape(C, 9, C)
            w[0:C, ci, :, 0:C] = fs0
            w[C:128, ci, :, C:128] = fs1
        in_maps.append({"xp": xp, "w": w, "cst": gb, "eye": eye})
    return in_maps


def _run(in_maps, trace=False):
    nc = _get_nc()
    return run_bass_kernel_spmd(nc, in_maps, core_ids=list(range(N_CORES)),
                                trace=trace)


def _gather(res):
    out = np.empty((B, C, H, W), np.float32)
    for i in range(N_CORES):
        o = np.asarray(res.results[i]["out"], dtype=np.float32)
        out[SPC * i] = o[0:C]
        out[SPC * i + 1] = o[C:128]
    return out


def kernel(x, filters1, filters2, gamma1, beta1, gamma2, beta2):
    in_maps = _pack_inputs(x, filters1, filters2, gamma1, beta1, gamma2, beta2)
    res = _run(in_maps, trace=False)
    return _gather(res)
